# revision 16
# baseline (speedup 1.0000x reference)
"""CPCNet forward on 8 Trainium2 NeuronCores (Bass/Tile).  v8

Data-parallel over batch: each core does 16 of 128 batch elements
(embed GEMM -> GRU over 16 context windows -> bilinear scoring),
parameters replicated, no collectives.

Input staging (host, inside kernel(), like the W_embed pre-chunk/cast):
X ships to device DRAM transposed AND pass-packed in bf16: for a pass
covering window-columns [c0, c0+PC), XT[q, j*PC + c] = X^T[k=j*128+q,
row=c0+c], so any run of whole k-chunks is per-partition CONTIGUOUS.
Four passes: 512-col passes over block 0 (Xc+Xp) and block 1 (nb0-1),
then 1024-col passes over blocks 2-3 and 4-5.  Tiles span 8 k-chunks ->
1-2 MB DMAs with 8-16 KB contiguous descriptors, alternating gpsimd
SWDGE / sync HWDGE issue queues: ~330 GB/s HBM streaming.

Device kernel = streaming GEMM at the bf16 HBM roofline (W chunk
stationary, accumulate E^T per block in PSUM, bias-evac to
ET[100, 3072]).  No on-chip transposes.

The early tiny pass 0a gets Ec evacuated ~30us in, so the serial GRU
chain (DVE elementwise + ACT sigmoid/tanh; gpsimd does nothing but DMA
issue) starts early and its 16 steps spread at >= chain-latency spacing
across passes 0b-1 -- the PE stream never waits on it.  Bilinear
A+products hide in pass 2; the tail is the nb6-9 products and the
pipelined float32r ones-matmul reduction.
"""

import numpy as np

import concourse.bacc as bacc
import concourse.mybir as mybir
import concourse.tile as tile
from concourse.bass_utils import run_bass_kernel_spmd

N_CORES = 8
BC = 16          # batch per core
NE = 16          # context windows (gru seq len)
NB = 10          # negative samples
CT = 8400        # flattened window (21*400)
E = 100          # embed dim == gru hidden
ROWS = BC * NE * (2 + NB)   # 3072 rows per core
NCHUNK = 66                 # 8448 / 128 k-chunks (last 48 rows zero-pad)
CTP = NCHUNK * 128          # 8448
TC = 8                      # k-chunks per tile (9 tiles: 8x8+2)

F32 = mybir.dt.float32
BF16 = mybir.dt.bfloat16

# (name, col offset, pass width) -- pass 0 split so Ec evacuates early
PASSES = [("XT0a", 0, 512), ("XT0b", 512, 512),
          ("XT1", 1024, 1024), ("XT2", 2048, 1024)]


def _emit(nc, tc_, ctx):
    XTs = {name: nc.dram_tensor(name, [128, NCHUNK * pc], BF16,
                                kind="ExternalInput").ap()
           for name, _, pc in PASSES}
    Wemb = nc.dram_tensor("Wemb", [128, NCHUNK * E], BF16,
                          kind="ExternalInput").ap()
    bemb = nc.dram_tensor("bemb", [E, 1], F32, kind="ExternalInput").ap()
    WihT = nc.dram_tensor("WihT", [E, 300], F32, kind="ExternalInput").ap()
    WhhT = nc.dram_tensor("WhhT", [E, 300], F32, kind="ExternalInput").ap()
    bias4 = nc.dram_tensor("bias4", [E, 4], F32, kind="ExternalInput").ap()
    Wbil = nc.dram_tensor("Wbil", [E, NE * E], F32, kind="ExternalInput").ap()
    ones = nc.dram_tensor("ones", [E, 1], mybir.dt.float32r,
                          kind="ExternalInput").ap()
    out_d = nc.dram_tensor("out", [1, NE * BC * (NB + 1)], F32,
                           kind="ExternalOutput").ap()

    P = ctx.enter_context  # pools

    const = P(tc_.tile_pool(name="const", bufs=1))
    xt0p = P(tc_.tile_pool(name="xt0", bufs=4))
    xt1p = P(tc_.tile_pool(name="xt1", bufs=4))
    psE = P(tc_.tile_pool(name="psE", bufs=2, space="PSUM"))
    psS = P(tc_.tile_pool(name="psS", bufs=1, space="PSUM"))
    small = P(tc_.tile_pool(name="small", bufs=2))

    # ---- persistent SBUF ----
    # W first on the (empty) sync queue: ready before the first X tile
    W_sb = const.tile([128, NCHUNK * E], BF16)
    nc.sync.dma_start(W_sb[:], Wemb[:])
    bemb_sb = const.tile([E, 1], F32)
    nc.scalar.dma_start(bemb_sb[:], bemb[:])
    WihT_sb = const.tile([E, 300], F32)
    nc.scalar.dma_start(WihT_sb[:], WihT[:])
    WhhT_sb = const.tile([E, 300], F32)
    nc.scalar.dma_start(WhhT_sb[:], WhhT[:])
    bias4_sb = const.tile([E, 4], F32)
    nc.scalar.dma_start(bias4_sb[:], bias4[:])
    Wbil_sb = const.tile([E, NE * E], F32)
    nc.scalar.dma_start(Wbil_sb[:], Wbil[:])
    ones_sb = const.tile([E, 1], mybir.dt.float32r)
    nc.scalar.dma_start(ones_sb[:], ones[:])

    ET = const.tile([E, ROWS], F32)                # all embeddings, transposed
    gi_sb = const.tile([E, NE * 3 * BC], F32)      # preacts, [s][r|z|n] blocks
    h = const.tile([E, BC], F32)                   # GRU hidden state (h^T)
    tmp_all = const.tile([E, NE * BC * (NB + 1)], mybir.dt.float32r)
    out_sb = const.tile([1, NE * BC * (NB + 1)], F32)

    A_sb = const.tile([E, NE * BC], F32)           # bilinear A, persisted
    gi_v = gi_sb.rearrange("e (s g b) -> e s g b", s=NE, g=3)
    tmp_v = tmp_all.rearrange("e (s b p) -> e s b p", s=NE, b=BC)
    Eb_v = ET[:, 512:ROWS].rearrange("e (nb s b) -> e nb s b", nb=NB, s=NE)

    def bil_part(s0, s1):
        # A_s = W_bil[s].T @ h^T plus the Ep and nb0-5 score products
        # (blocks 0-3, evacuated by end of pass 1) -- spread over pass 2;
        # the nb6-9 products run as the tail.
        for s in range(s0, s1):
            Ap = psS.tile([E, BC], F32, tag="bilA", name="Ap", bufs=2)
            nc.tensor.matmul(Ap[:, :], Wbil_sb[:, s * E:(s + 1) * E], h[:],
                             start=True, stop=True)
            nc.scalar.copy(A_sb[:, s * BC:(s + 1) * BC], Ap[:])
            nc.vector.tensor_mul(tmp_v[:, s, :, 0],
                                 ET[:, NE * BC + s * BC: NE * BC + (s + 1) * BC],
                                 Ap[:])
            nc.vector.tensor_mul(
                tmp_v[:, s, :, 1:7].rearrange("e b p -> e p b"),
                Eb_v[:, 0:6, s, :],
                Ap[:].unsqueeze(1).broadcast_to([E, 6, BC]))

    def gru_init():
        nc.vector.memset(h[:], 0.0)
        for g in range(3):
            gp = psS.tile([E, NE * BC], F32, tag="sp0", name="gp")
            nc.tensor.matmul(gp[:, :], WihT_sb[:, g * E:(g + 1) * E],
                             ET[:, 0:NE * BC], start=True, stop=True)
            nc.scalar.add(gi_v[:, :, g, :],
                          gp.rearrange("e (s b) -> e s b", s=NE),
                          bias4_sb[:, g:g + 1])

    def gru_step(s):
        # serial chain on DVE (elementwise) + ACT (sigmoid/tanh) only;
        # gpsimd stays free for DMA issue so the chain never stalls
        # behind buffer-full DMA waits
        c0 = s * 3 * BC
        gh = psS.tile([E, 3 * BC], F32, tag="sp1", name="gh")
        for g in range(3):
            nc.tensor.matmul(gh[:, g * BC:(g + 1) * BC],
                             WhhT_sb[:, g * E:(g + 1) * E], h[:],
                             start=True, stop=True)
        rzt = small.tile([E, 2 * BC], F32, tag="rzt", name="rzt")
        nc.vector.tensor_add(rzt[:], gh[:, 0:2 * BC], gi_sb[:, c0:c0 + 2 * BC])
        rz = small.tile([E, 2 * BC], F32, tag="rz", name="rz")
        nc.scalar.activation(rz[:], rzt[:],
                             mybir.ActivationFunctionType.Sigmoid)
        hn = small.tile([E, BC], F32, tag="hn", name="hn")
        nc.vector.tensor_scalar_add(hn[:], gh[:, 2 * BC:3 * BC],
                                    bias4_sb[:, 3:4])  # gh_n + b_hn
        t1 = small.tile([E, BC], F32, tag="t1", name="t1")
        nc.vector.tensor_mul(t1[:], rz[:, 0:BC], hn[:])
        t2 = small.tile([E, BC], F32, tag="t2", name="t2")
        nc.vector.tensor_add(t2[:], t1[:], gi_sb[:, c0 + 2 * BC:c0 + 3 * BC])
        n = small.tile([E, BC], F32, tag="n", name="n")
        nc.scalar.activation(n[:], t2[:], mybir.ActivationFunctionType.Tanh)
        d = small.tile([E, BC], F32, tag="d", name="d")
        nc.vector.tensor_sub(d[:], h[:], n[:])
        zd = small.tile([E, BC], F32, tag="zd", name="zd")
        nc.vector.tensor_mul(zd[:], rz[:, BC:2 * BC], d[:])
        nc.vector.tensor_add(h[:], n[:], zd[:])    # h = n + z*(h-n)

    # serial-work schedule at half-tile slots: the GRU chain latency is
    # ~3us/step, so emission spacing must stay >= that or the in-order
    # PE queue stalls the embed stream behind a waiting gh matmul
    def interleave(pi, jt, u):
        if pi == 1 and u == 0 and jt in (1, 3, 5, 7):
            gru_step((jt - 1) // 2)                # steps 0-3
        elif pi == 2 and u in (0, 4):
            k = 2 * jt + (0 if u == 0 else 1)
            if k < 12:
                gru_step(4 + k)                    # steps 4-15
        elif pi == 3 and u == 0 and jt in (1, 3, 5, 7):
            q = (jt - 1) // 2
            bil_part(4 * q, 4 * q + 4)

    # ---- embed: 4 passes over k ----
    NTILE = (NCHUNK + TC - 1) // TC                # 9
    for pi, (name, c0, pc) in enumerate(PASSES):
        nmm2 = pc // 512                           # MMs per chunk (1 or 2)
        ets = [psE.tile([E, 512], F32, tag=f"et{i}", name=f"et{i}")
               for i in range(nmm2)]
        pool = xt0p if pc == 512 else xt1p
        for jt in range(NTILE):
            j0 = jt * TC
            jn = min(TC, NCHUNK - j0)
            xt = pool.tile([128, TC * pc], BF16, name="xt")
            eng = nc.gpsimd if jt % 2 == 0 else nc.sync
            eng.dma_start(xt[:, 0:jn * pc],
                          XTs[name][:, j0 * pc:(j0 + jn) * pc])
            for u in range(jn):
                interleave(pi, jt, u)
                j = j0 + u
                for i in range(nmm2):
                    nc.tensor.matmul(
                        ets[i][:, :], W_sb[:, j * E:(j + 1) * E],
                        xt[:, u * pc + i * 512:u * pc + (i + 1) * 512],
                        start=(j == 0), stop=(j == NCHUNK - 1),
                        skip_group_check=True)
        for i in range(nmm2):
            nc.scalar.add(ET[:, c0 + i * 512:c0 + (i + 1) * 512],
                          ets[i][:, :], bemb_sb[:, 0:1])
        # gi preacts as soon as block 0 (Ec, first 256 cols) is done
        if pi == 0:
            gru_init()

    # ---- tail: nb6-9 products (blocks 4-5) + ones-matmul reduction ----
    for s in range(NE):
        nc.vector.tensor_mul(
            tmp_v[:, s, :, 7:NB + 1].rearrange("e b p -> e p b"),
            Eb_v[:, 6:10, s, :],
            A_sb[:, s * BC:(s + 1) * BC].unsqueeze(1).broadcast_to([E, 4, BC]))
    TOT = NE * BC * (NB + 1)
    for cc in range(0, TOT, 512):
        w = min(512, TOT - cc)
        # reuse the bilA psum slots (free by now): bufs=2 pipelines the
        # MM -> copy ladder instead of serializing at ~1.3us each
        rp = psS.tile([1, 512], F32, tag="bilA", name="rp", bufs=2)
        nc.tensor.matmul(rp[0:1, 0:w], ones_sb[:, 0:1], tmp_all[:, cc:cc + w],
                         start=True, stop=True)
        nc.scalar.copy(out_sb[:, cc:cc + w], rp[0:1, 0:w])
    nc.sync.dma_start(out_d[:], out_sb[:])


def build():
    import contextlib
    nc = bacc.Bacc("TRN2", target_bir_lowering=False, debug=False,
                   enable_asserts=False, num_devices=N_CORES)
    with tile.TileContext(nc) as tc_:
        with contextlib.ExitStack() as ctx:
            _emit(nc, tc_, ctx)
    nc.compile()
    return nc


_NC = None


def make_in_maps(Xc, Xp, Xb, W_embed, b_embed, W_ih, W_hh, b_ih, b_hh, W_bil):
    import ml_dtypes
    B = Xc.shape[0]
    BF = ml_dtypes.bfloat16
    Xc_b = np.asarray(Xc, np.float32).reshape(B, NE, CT).astype(BF)
    Xp_b = np.asarray(Xp, np.float32).reshape(B, NE, CT).astype(BF)
    Xb_b = np.asarray(Xb, np.float32).reshape(B, NE, NB, CT).astype(BF)

    W_embed = np.ascontiguousarray(W_embed, np.float32)
    W_ch = np.zeros((128, NCHUNK * E), np.float32)
    for j in range(NCHUNK):
        kj = min(128, CT - j * 128)
        W_ch[:kj, j * E:(j + 1) * E] = W_embed[j * 128:j * 128 + kj]
    W_ch = W_ch.astype(BF)
    bemb = np.ascontiguousarray(b_embed, np.float32).reshape(E, 1)
    WihT = np.ascontiguousarray(W_ih.T, np.float32)          # [100, 300]
    WhhT = np.ascontiguousarray(W_hh.T, np.float32)
    bias4 = np.stack([b_ih[0:E] + b_hh[0:E],
                      b_ih[E:2 * E] + b_hh[E:2 * E],
                      b_ih[2 * E:3 * E],
                      b_hh[2 * E:3 * E]], axis=1).astype(np.float32)
    Wbil_r = np.ascontiguousarray(
        np.transpose(W_bil, (1, 0, 2)).reshape(E, NE * E), np.float32)
    ones = np.ones((E, 1), np.float32)

    shared = dict(Wemb=W_ch, bemb=bemb, WihT=WihT, WhhT=WhhT,
                  bias4=bias4, Wbil=Wbil_r, ones=ones)
    in_maps = []
    for c in range(N_CORES):
        sl = slice(c * BC, (c + 1) * BC)
        # rows in ET column order: Xc (s,b) | Xp (s,b) | Xb (nb,s,b)
        A = np.zeros((ROWS, CTP), BF)
        A[0:256, 0:CT] = Xc_b[sl].transpose(1, 0, 2).reshape(256, CT)
        A[256:512, 0:CT] = Xp_b[sl].transpose(1, 0, 2).reshape(256, CT)
        A[512:, 0:CT] = Xb_b[sl].transpose(2, 1, 0, 3).reshape(2560, CT)
        m = dict(shared)
        for name, c0, pc in PASSES:
            m[name] = np.ascontiguousarray(
                A[c0:c0 + pc]
                .reshape(pc, NCHUNK, 128)
                .transpose(2, 1, 0)
                .reshape(128, NCHUNK * pc))
        in_maps.append(m)
    return in_maps


def gather(results):
    outs = []
    for c in range(N_CORES):
        o = results[c]["out"].reshape(NE, BC, NB + 1)       # [s, b, p]
        outs.append(np.transpose(o, (1, 0, 2)))             # [b, s, p]
    return np.concatenate(outs, axis=0).astype(np.float32)  # [128, 16, 11]


def kernel(Xc, Xp, Xb, W_embed, b_embed, W_ih, W_hh, b_ih, b_hh, W_bil):
    global _NC
    if _NC is None:
        _NC = build()
    in_maps = make_in_maps(Xc, Xp, Xb, W_embed, b_embed, W_ih, W_hh,
                           b_ih, b_hh, W_bil)
    res = run_bass_kernel_spmd(_NC, in_maps, core_ids=list(range(N_CORES)))
    return gather(res.results)
